# revision 16
# baseline (speedup 1.0000x reference)
"""TRN2 Bass kernel for nn_BlockLinear: per token t, x_t [32,128] ->
P(P(x_t@w1)@w2) where P(Y) = reshape(Y.T, (32,128)).

v3 strategy (data-parallel over 8 NeuronCores, 4096 tokens/core):
  - All wire traffic in bf16 (halves HBM bytes; rel-err budget 2e-2 allows it).
  - Host pre-transposes x to XT[k, (tau,b,h)] with t = 2*tau + h so the
    contraction dim k is on partitions at DMA time -> NO on-device input
    transpose.  Host also absorbs the final P permutation on download ->
    NO on-device output transpose.  Only the mid-stage P runs on-device.
  - Weight columns are permuted (w*p = w*[:, perm], perm[32a+i] = 4i+a) so
    the mid-stage P becomes a 32x32 diagonal-block transpose (DVE native).
    The adjacent-token pair (h=0/1) of a given (b) occupies one aligned
    bf16 pair and travels to the same destination partition, so the DVE
    transpose runs on an int32 view -> half the DVE cycles.
  - Per 256-token chunk: DMA in -> 16x matmul(w1p) -> 8x cast-evac PSUM
    f32 -> SBUF bf16 (split Act/DVE; GpSimd cannot touch PSUM and its
    casts are ~3.5us/tile) -> 2x DVE int32 block-transpose -> 16x
    matmul(w2p) -> 8x cast-evac (Act/DVE) -> DMA out.  Chunk-level
    software pipeline (stage1 of chunk c+1 before stage2 of chunk c)
    keeps PE warm while DVE/Act drain chunk c.
"""
import numpy as np
import ml_dtypes
from contextlib import ExitStack

import concourse.bass as bass
from concourse import bacc
import concourse.tile as tile
from concourse import mybir
from concourse.bass_utils import run_bass_kernel_spmd

F32 = mybir.dt.float32
BF16 = mybir.dt.bfloat16
I32 = mybir.dt.int32

N_CORES = 8
TOK_PER_CORE = 4096
CHUNK_TOK = 256          # tokens per chunk; free dim = 32*256 = 8192
N = 4096                 # elems per token

# engine split for the 8+8 PSUM->SBUF cast-evac tiles per chunk
# (measured: Act ~1236ns/tile, DVE cast ~1390ns/tile + 2x2714ns transposes)
EVAC1 = ("act", "dve", "act", "act", "dve", "act", "act", "dve")  # 5A/3D
EVAC2 = ("act", "dve", "act", "act", "act", "dve", "act", "act")  # 6A/2D


def _perm():
    p = np.zeros(128, np.int64)
    for a in range(4):
        for i in range(32):
            p[32 * a + i] = 4 * i + a
    return p


def _f32_to_bf16_u16(a):
    """Round-to-nearest-even f32 -> bf16 bit pattern (uint16)."""
    u = np.ascontiguousarray(a, np.float32).view(np.uint32)
    r = ((u.astype(np.uint64) + 0x7FFF + ((u >> 16) & 1)) >> 16).astype(np.uint16)
    return r


def _u16_to_f32(u):
    return (u.astype(np.uint32) << 16).view(np.float32)


def build_nc(ntok):
    nchunks = ntok // CHUNK_TOK
    FD = CHUNK_TOK * 32  # free-dim elems per chunk
    nc = bacc.Bacc("TRN2", target_bir_lowering=False, debug=False)
    X = nc.dram_tensor("xt", [128, ntok * 32], BF16, kind="ExternalInput").ap()
    W1 = nc.dram_tensor("w1p", [128, 128], BF16, kind="ExternalInput").ap()
    W2 = nc.dram_tensor("w2p", [128, 128], BF16, kind="ExternalInput").ap()
    OUT = nc.dram_tensor("out", [128, ntok * 32], BF16, kind="ExternalOutput").ap()

    def evac(kind, dst, src):
        if kind == "act":
            nc.scalar.copy(dst, src)
        else:
            nc.vector.tensor_copy(dst, src)

    with tile.TileContext(nc) as tc, ExitStack() as ctx:
        wpool = ctx.enter_context(tc.tile_pool(name="w", bufs=1))
        xtp = ctx.enter_context(tc.tile_pool(name="xtp", bufs=2))
        y1p = ctx.enter_context(tc.tile_pool(name="y1p", bufs=2))
        m2p = ctx.enter_context(tc.tile_pool(name="m2p", bufs=2))
        obp = ctx.enter_context(tc.tile_pool(name="obp", bufs=2))
        psp = ctx.enter_context(tc.tile_pool(name="psp", bufs=2, space="PSUM"))

        w1_sb = wpool.tile([128, 128], BF16)
        w2_sb = wpool.tile([128, 128], BF16)
        nc.sync.dma_start(w1_sb[:], W1[:])
        nc.sync.dma_start(w2_sb[:], W2[:])

        m_tiles = {}
        y_tiles = {}
        ob_tiles = {}
        NQ = FD // 1024

        for c in range(nchunks + 1):
            # q-level software pipeline: stage1(c) and stage2(c-1) interleave
            # so PE alternates mm1/mm2 and evacs spread evenly across engines.
            if c < nchunks:
                xt = xtp.tile([128, FD], BF16, tag="xt")
                nc.sync.dma_start(xt[:], X[:, c * FD:(c + 1) * FD])
                y_tiles[c] = y1p.tile([128, FD], BF16, tag="y1e", name="y1e")
            if c >= 1:
                ob_tiles[c - 1] = obp.tile([128, FD], BF16, tag="ob", name="ob")
            for q in range(NQ):
                if c < nchunks:
                    y1e = y_tiles[c]
                    ps = psp.tile([128, 1024], F32, tag="a")
                    nc.tensor.matmul(ps[:, 0:512], w1_sb[:],
                                     xt[:, bass.ts(2 * q, 512)],
                                     start=True, stop=True)
                    nc.tensor.matmul(ps[:, 512:1024], w1_sb[:],
                                     xt[:, bass.ts(2 * q + 1, 512)],
                                     start=True, stop=True)
                    evac(EVAC1[q % 8], y1e[:, bass.ts(q, 1024)], ps[:])
                if c >= 1:
                    m2 = m_tiles[c - 1]
                    ob = ob_tiles[c - 1]
                    ps = psp.tile([128, 1024], F32, tag="b")
                    nc.tensor.matmul(ps[:, 0:512], w2_sb[:],
                                     m2[:, bass.ts(2 * q, 512)],
                                     start=True, stop=True)
                    nc.tensor.matmul(ps[:, 512:1024], w2_sb[:],
                                     m2[:, bass.ts(2 * q + 1, 512)],
                                     start=True, stop=True)
                    evac(EVAC2[q % 8], ob[:, bass.ts(q, 1024)], ps[:])
                # mid-stage transpose once all stage-1 evacs have landed;
                # whole-tile bitcast APs keep dependency ranges exact.
                if c < nchunks and q == NQ - 1:
                    m_tiles[c] = m2p.tile([128, FD], BF16, tag="m2", name="m2")
                    nc.vector.transpose(m_tiles[c][:].bitcast(I32),
                                        y_tiles[c][:].bitcast(I32))
            if c >= 1:
                nc.sync.dma_start(OUT[:, (c - 1) * FD:c * FD], ob_tiles[c - 1][:])
                m_tiles.pop(c - 1)
                ob_tiles.pop(c - 1)
                y_tiles.pop(c - 1, None)

    if not nc.is_finalized():
        nc.finalize()
    return nc


_NC_CACHE = {}


def _get_nc(ntok):
    if ntok not in _NC_CACHE:
        _NC_CACHE[ntok] = build_nc(ntok)
    return _NC_CACHE[ntok]


def prepare_in_maps(x, w1, w2):
    """Host-side shard + layout transform. Returns (in_maps, ntok)."""
    xf = np.ascontiguousarray(x, dtype=np.float32).reshape(-1, N)
    ntok_total = xf.shape[0]
    assert ntok_total % N_CORES == 0
    ntok = ntok_total // N_CORES

    perm = _perm()
    w1p = _f32_to_bf16_u16(np.ascontiguousarray(w1, np.float32)[:, perm])
    w2p = _f32_to_bf16_u16(np.ascontiguousarray(w2, np.float32)[:, perm])
    w1p = w1p.view(ml_dtypes.bfloat16)
    w2p = w2p.view(ml_dtypes.bfloat16)

    xu = _f32_to_bf16_u16(xf)  # [T, 4096] u16
    in_maps = []
    for i in range(N_CORES):
        xc = xu[i * ntok:(i + 1) * ntok].reshape(ntok // 2, 2, 32, 128)
        # XT[k, tau*64 + 2b + h] = x[2 tau + h, 128 b + k]
        xt = np.ascontiguousarray(xc.transpose(3, 0, 2, 1)).reshape(128, ntok * 32)
        in_maps.append({
            "xt": xt.view(ml_dtypes.bfloat16),
            "w1p": w1p, "w2p": w2p,
        })
    return in_maps, ntok


def postprocess(results, ntok, lead):
    """Gather per-core OT [128, ntok*32] bf16 -> full f32 output."""
    ntok_total = ntok * N_CORES
    out = np.empty((ntok_total, N), np.float32)
    for i in range(N_CORES):
        ot = np.asarray(results[i]["out"]).view(np.uint16)
        # out[2 tau + h, 128 i2 + 32 a2 + b2] = OT[32 a2 + i2, tau*64 + 2 b2 + h]
        ot = ot.reshape(4, 32, ntok // 2, 32, 2)      # [a2, i2, tau, b2, h]
        oc = ot.transpose(2, 4, 1, 0, 3).reshape(ntok, N)
        out[i * ntok:(i + 1) * ntok] = _u16_to_f32(np.ascontiguousarray(oc))
    return out.reshape(*lead, N)


def kernel(x, w1, w2):
    """x [8, 4096, 4096] f32; w1, w2 [128, 128] f32 -> [8, 4096, 4096] f32."""
    lead = x.shape[:-1]
    in_maps, ntok = prepare_in_maps(x, w1, w2)
    nc = _get_nc(ntok)
    res = run_bass_kernel_spmd(nc, in_maps, list(range(N_CORES)))
    return postprocess(res.results, ntok, lead)


# revision 18
# speedup vs baseline: 1.2612x; 1.2612x over previous
"""TRN2 Bass kernel for nn_BlockLinear: per token t, x_t [32,128] ->
P(P(x_t@w1)@w2) where P(Y) = reshape(Y.T, (32,128)).

v3 strategy (data-parallel over 8 NeuronCores, 4096 tokens/core):
  - All wire traffic in bf16 (halves HBM bytes; rel-err budget 2e-2 allows it).
  - Host pre-transposes x to XT[k, (tau,b,h)] with t = 2*tau + h so the
    contraction dim k is on partitions at DMA time -> NO on-device input
    transpose.  Host also absorbs the final P permutation on download ->
    NO on-device output transpose.  Only the mid-stage P runs on-device.
  - Weight columns are permuted (w*p = w*[:, perm], perm[32a+i] = 4i+a) so
    the mid-stage P becomes a 32x32 diagonal-block transpose (DVE native).
    The adjacent-token pair (h=0/1) of a given (b) occupies one aligned
    bf16 pair and travels to the same destination partition, so the DVE
    transpose runs on an int32 view -> half the DVE cycles.
  - Per 256-token chunk: DMA in -> 16x matmul(w1p) -> 8x cast-evac PSUM
    f32 -> SBUF bf16 (split Act/DVE; GpSimd cannot touch PSUM and its
    casts are ~3.5us/tile) -> 2x DVE int32 block-transpose -> 16x
    matmul(w2p) -> 8x cast-evac (Act/DVE) -> DMA out.  Chunk-level
    software pipeline (stage1 of chunk c+1 before stage2 of chunk c)
    keeps PE warm while DVE/Act drain chunk c.
"""
import numpy as np
import ml_dtypes
from contextlib import ExitStack

import concourse.bass as bass
from concourse import bacc
import concourse.tile as tile
from concourse import mybir
from concourse.bass_utils import run_bass_kernel_spmd

F32 = mybir.dt.float32
BF16 = mybir.dt.bfloat16
I32 = mybir.dt.int32

N_CORES = 8
TOK_PER_CORE = 4096
CHUNK_TOK = 256          # tokens per chunk; free dim = 32*256 = 8192
N = 4096                 # elems per token

# engine split for the 8+8 PSUM->SBUF cast-evac tiles per chunk
# (measured: Act ~1236ns/tile, DVE cast ~1390ns/tile + 2x2714ns transposes)
EVAC1 = ("act", "dve", "act", "act", "dve", "act", "act", "dve")  # 5A/3D
EVAC2 = ("act", "dve", "act", "act", "act", "dve", "act", "act")  # 6A/2D


def _perm():
    p = np.zeros(128, np.int64)
    for a in range(4):
        for i in range(32):
            p[32 * a + i] = 4 * i + a
    return p


def _f32_to_bf16_u16(a):
    """Round-to-nearest-even f32 -> bf16 bit pattern (uint16)."""
    u = np.ascontiguousarray(a, np.float32).view(np.uint32)
    r = ((u.astype(np.uint64) + 0x7FFF + ((u >> 16) & 1)) >> 16).astype(np.uint16)
    return r


def _u16_to_f32(u):
    return (u.astype(np.uint32) << 16).view(np.float32)


def build_nc(ntok):
    nchunks = ntok // CHUNK_TOK
    FD = CHUNK_TOK * 32  # free-dim elems per chunk
    nc = bacc.Bacc("TRN2", target_bir_lowering=False, debug=False)
    X = nc.dram_tensor("xt", [128, ntok * 32], BF16, kind="ExternalInput").ap()
    W1 = nc.dram_tensor("w1p", [128, 128], BF16, kind="ExternalInput").ap()
    W2 = nc.dram_tensor("w2p", [128, 128], BF16, kind="ExternalInput").ap()
    OUT = nc.dram_tensor("out", [128, ntok * 32], BF16, kind="ExternalOutput").ap()

    def evac(kind, dst, src):
        if kind == "act":
            nc.scalar.copy(dst, src)
        else:
            nc.vector.tensor_copy(dst, src)

    with tile.TileContext(nc) as tc, ExitStack() as ctx:
        wpool = ctx.enter_context(tc.tile_pool(name="w", bufs=1))
        xtp = ctx.enter_context(tc.tile_pool(name="xtp", bufs=2))
        y1p = ctx.enter_context(tc.tile_pool(name="y1p", bufs=4))
        m2p = ctx.enter_context(tc.tile_pool(name="m2p", bufs=4))
        obp = ctx.enter_context(tc.tile_pool(name="obp", bufs=2))
        psp = ctx.enter_context(tc.tile_pool(name="psp", bufs=2, space="PSUM"))

        w1_sb = wpool.tile([128, 128], BF16)
        w2_sb = wpool.tile([128, 128], BF16)
        nc.sync.dma_start(w1_sb[:], W1[:])
        nc.sync.dma_start(w2_sb[:], W2[:])

        m_tiles = {}
        y_tiles = {}
        ob_tiles = {}
        NQ = FD // 1024
        HFD = FD // 2

        def mm_pair(ps, w_sb, src, q):
            nc.tensor.matmul(ps[:, 0:512], w_sb[:],
                             src[:, bass.ts(2 * q, 512)], start=True, stop=True)
            nc.tensor.matmul(ps[:, 512:1024], w_sb[:],
                             src[:, bass.ts(2 * q + 1, 512)],
                             start=True, stop=True)

        for c in range(nchunks + 1):
            # q-level software pipeline: stage1(c) and stage2(c-1) interleave
            # in pairs (halves LDWEIGHTS thrash) so evacs spread across engines.
            if c < nchunks:
                xt = xtp.tile([128, FD], BF16, tag="xt")
                nc.sync.dma_start(xt[:], X[:, c * FD:(c + 1) * FD])
                # per-half tiles: transposes use whole-tile bitcast APs only
                y_tiles[c] = [y1p.tile([128, HFD], BF16, tag="y1e", name="y1e")
                              for _ in range(2)]
            if c >= 1:
                ob_tiles[c - 1] = obp.tile([128, FD], BF16, tag="ob", name="ob")
            for qq in range(NQ // 2):
                for q in (2 * qq, 2 * qq + 1):
                    if c < nchunks:
                        y1e = y_tiles[c][q // (NQ // 2)]
                        ps = psp.tile([128, 1024], F32, tag="a")
                        mm_pair(ps, w1_sb, xt, q)
                        evac(EVAC1[q % 8],
                             y1e[:, bass.ts(q % (NQ // 2), 1024)], ps[:])
                for q in (2 * qq, 2 * qq + 1):
                    if c >= 1:
                        m2 = m_tiles[c - 1][q // (NQ // 2)]
                        ob = ob_tiles[c - 1]
                        ps = psp.tile([128, 1024], F32, tag="b")
                        mm_pair(ps, w2_sb, m2, q % (NQ // 2))
                        evac(EVAC2[q % 8], ob[:, bass.ts(q, 1024)], ps[:])
                # transpose each half as soon as its stage-1 evacs land
                if c < nchunks and qq in (NQ // 4 - 1, NQ // 2 - 1):
                    hh = 0 if qq == NQ // 4 - 1 else 1
                    if hh == 0:
                        m_tiles[c] = [m2p.tile([128, HFD], BF16, tag="m2",
                                               name="m2") for _ in range(2)]
                    nc.vector.transpose(m_tiles[c][hh][:].bitcast(I32),
                                        y_tiles[c][hh][:].bitcast(I32))
            if c >= 1:
                nc.sync.dma_start(OUT[:, (c - 1) * FD:c * FD], ob_tiles[c - 1][:])
                m_tiles.pop(c - 1)
                ob_tiles.pop(c - 1)
                y_tiles.pop(c - 1, None)

    if not nc.is_finalized():
        nc.finalize()
    return nc


_NC_CACHE = {}


def _get_nc(ntok):
    if ntok not in _NC_CACHE:
        _NC_CACHE[ntok] = build_nc(ntok)
    return _NC_CACHE[ntok]


def prepare_in_maps(x, w1, w2):
    """Host-side shard + layout transform. Returns (in_maps, ntok)."""
    xf = np.ascontiguousarray(x, dtype=np.float32).reshape(-1, N)
    ntok_total = xf.shape[0]
    assert ntok_total % N_CORES == 0
    ntok = ntok_total // N_CORES

    perm = _perm()
    w1p = _f32_to_bf16_u16(np.ascontiguousarray(w1, np.float32)[:, perm])
    w2p = _f32_to_bf16_u16(np.ascontiguousarray(w2, np.float32)[:, perm])
    w1p = w1p.view(ml_dtypes.bfloat16)
    w2p = w2p.view(ml_dtypes.bfloat16)

    xu = _f32_to_bf16_u16(xf)  # [T, 4096] u16
    in_maps = []
    for i in range(N_CORES):
        xc = xu[i * ntok:(i + 1) * ntok].reshape(ntok // 2, 2, 32, 128)
        # XT[k, tau*64 + 2b + h] = x[2 tau + h, 128 b + k]
        xt = np.ascontiguousarray(xc.transpose(3, 0, 2, 1)).reshape(128, ntok * 32)
        in_maps.append({
            "xt": xt.view(ml_dtypes.bfloat16),
            "w1p": w1p, "w2p": w2p,
        })
    return in_maps, ntok


def postprocess(results, ntok, lead):
    """Gather per-core OT [128, ntok*32] bf16 -> full f32 output."""
    ntok_total = ntok * N_CORES
    out = np.empty((ntok_total, N), np.float32)
    for i in range(N_CORES):
        ot = np.asarray(results[i]["out"]).view(np.uint16)
        # out[2 tau + h, 128 i2 + 32 a2 + b2] = OT[32 a2 + i2, tau*64 + 2 b2 + h]
        ot = ot.reshape(4, 32, ntok // 2, 32, 2)      # [a2, i2, tau, b2, h]
        oc = ot.transpose(2, 4, 1, 0, 3).reshape(ntok, N)
        out[i * ntok:(i + 1) * ntok] = _u16_to_f32(np.ascontiguousarray(oc))
    return out.reshape(*lead, N)


def kernel(x, w1, w2):
    """x [8, 4096, 4096] f32; w1, w2 [128, 128] f32 -> [8, 4096, 4096] f32."""
    lead = x.shape[:-1]
    in_maps, ntok = prepare_in_maps(x, w1, w2)
    nc = _get_nc(ntok)
    res = run_bass_kernel_spmd(nc, in_maps, list(range(N_CORES)))
    return postprocess(res.results, ntok, lead)


# revision 22
# speedup vs baseline: 1.2752x; 1.0111x over previous
"""TRN2 Bass kernel for nn_BlockLinear: per token t, x_t [32,128] ->
P(P(x_t@w1)@w2) where P(Y) = reshape(Y.T, (32,128)).

v3 strategy (data-parallel over 8 NeuronCores, 4096 tokens/core):
  - All wire traffic in bf16 (halves HBM bytes; rel-err budget 2e-2 allows it).
  - Host pre-transposes x to XT[k, (tau,b,h)] with t = 2*tau + h so the
    contraction dim k is on partitions at DMA time -> NO on-device input
    transpose.  Host also absorbs the final P permutation on download ->
    NO on-device output transpose.  Only the mid-stage P runs on-device.
  - Weight columns are permuted (w*p = w*[:, perm], perm[32a+i] = 4i+a) so
    the mid-stage P becomes a 32x32 diagonal-block transpose (DVE native).
    The adjacent-token pair (h=0/1) of a given (b) occupies one aligned
    bf16 pair and travels to the same destination partition, so the DVE
    transpose runs on an int32 view -> half the DVE cycles.
  - Per 256-token chunk: DMA in -> 16x matmul(w1p) -> 8x cast-evac PSUM
    f32 -> SBUF bf16 (split Act/DVE; GpSimd cannot touch PSUM and its
    casts are ~3.5us/tile) -> 2x DVE int32 block-transpose -> 16x
    matmul(w2p) -> 8x cast-evac (Act/DVE) -> DMA out.  Chunk-level
    software pipeline (stage1 of chunk c+1 before stage2 of chunk c)
    keeps PE warm while DVE/Act drain chunk c.
"""
import numpy as np
import ml_dtypes
from contextlib import ExitStack

import concourse.bass as bass
from concourse import bacc
import concourse.tile as tile
from concourse import mybir
from concourse.bass_utils import run_bass_kernel_spmd

F32 = mybir.dt.float32
BF16 = mybir.dt.bfloat16
I32 = mybir.dt.int32

N_CORES = 8
TOK_PER_CORE = 4096
CHUNK_TOK = 256          # tokens per chunk; free dim = 32*256 = 8192
N = 4096                 # elems per token

# engine split for the 8+8 PSUM->SBUF cast-evac tiles per chunk
# (measured: Act ~1035ns/tile, DVE cast ~1167ns/tile + 2x2271ns transposes);
# two tables alternated by chunk parity -> 10.5A/5.5D average
EVAC1A = ("act", "dve", "act", "act", "dve", "act", "act", "dve")  # 5A/3D
EVAC2A = ("act", "dve", "act", "act", "act", "dve", "act", "act")  # 6A/2D
EVAC1B = ("act", "dve", "act", "dve", "act", "act", "dve", "act")  # 5A/3D
EVAC2B = ("act", "dve", "act", "act", "dve", "act", "dve", "act")  # 5A/3D


def _perm():
    p = np.zeros(128, np.int64)
    for a in range(4):
        for i in range(32):
            p[32 * a + i] = 4 * i + a
    return p


def _f32_to_bf16_u16(a):
    """Round-to-nearest-even f32 -> bf16 bit pattern (uint16)."""
    u = np.ascontiguousarray(a, np.float32).view(np.uint32)
    r = ((u.astype(np.uint64) + 0x7FFF + ((u >> 16) & 1)) >> 16).astype(np.uint16)
    return r


def _u16_to_f32(u):
    return (u.astype(np.uint32) << 16).view(np.float32)


def build_nc(ntok):
    nchunks = ntok // CHUNK_TOK
    FD = CHUNK_TOK * 32  # free-dim elems per chunk
    nc = bacc.Bacc("TRN2", target_bir_lowering=False, debug=False)
    X = nc.dram_tensor("xt", [128, ntok * 32], BF16, kind="ExternalInput").ap()
    W1 = nc.dram_tensor("w1p", [128, 128], BF16, kind="ExternalInput").ap()
    W2 = nc.dram_tensor("w2p", [128, 128], BF16, kind="ExternalInput").ap()
    OUT = nc.dram_tensor("out", [128, ntok * 32], BF16, kind="ExternalOutput").ap()

    def evac(kind, dst, src):
        if kind == "act":
            nc.scalar.copy(dst, src)
        else:
            nc.vector.tensor_copy(dst, src)

    with tile.TileContext(nc) as tc, ExitStack() as ctx:
        wpool = ctx.enter_context(tc.tile_pool(name="w", bufs=1))
        xtp = ctx.enter_context(tc.tile_pool(name="xtp", bufs=3))
        y1p = ctx.enter_context(tc.tile_pool(name="y1p", bufs=4))
        m2p = ctx.enter_context(tc.tile_pool(name="m2p", bufs=4))
        obp = ctx.enter_context(tc.tile_pool(name="obp", bufs=3))
        psp = ctx.enter_context(tc.tile_pool(name="psp", bufs=2, space="PSUM"))

        w1_sb = wpool.tile([128, 128], BF16)
        w2_sb = wpool.tile([128, 128], BF16)
        nc.sync.dma_start(w1_sb[:], W1[:])
        nc.sync.dma_start(w2_sb[:], W2[:])

        m_tiles = {}
        y_tiles = {}
        ob_tiles = {}
        NQ = FD // 1024
        HFD = FD // 2

        def mm_pair(ps, w_sb, src, q):
            nc.tensor.matmul(ps[:, 0:512], w_sb[:],
                             src[:, bass.ts(2 * q, 512)], start=True, stop=True)
            nc.tensor.matmul(ps[:, 512:1024], w_sb[:],
                             src[:, bass.ts(2 * q + 1, 512)],
                             start=True, stop=True)

        for c in range(nchunks + 1):
            # q-level software pipeline: stage1(c) and stage2(c-1) interleave
            # in pairs (halves LDWEIGHTS thrash) so evacs spread across engines.
            if c < nchunks:
                xt = xtp.tile([128, FD], BF16, tag="xt")
                nc.sync.dma_start(xt[:], X[:, c * FD:(c + 1) * FD])
                # per-half tiles: transposes use whole-tile bitcast APs only
                y_tiles[c] = [y1p.tile([128, HFD], BF16, tag="y1e", name="y1e")
                              for _ in range(2)]
            if c >= 1:
                ob_tiles[c - 1] = obp.tile([128, FD], BF16, tag="ob", name="ob")
            for qq in range(NQ // 2):
                ev1 = EVAC1A if c % 2 == 0 else EVAC1B
                ev2 = EVAC2A if c % 2 == 0 else EVAC2B
                for q in (2 * qq, 2 * qq + 1):
                    if c < nchunks:
                        y1e = y_tiles[c][q // (NQ // 2)]
                        ps = psp.tile([128, 1024], F32, tag="a")
                        mm_pair(ps, w1_sb, xt, q)
                        evac(ev1[q % 8],
                             y1e[:, bass.ts(q % (NQ // 2), 1024)], ps[:])
                for q in (2 * qq, 2 * qq + 1):
                    if c >= 1:
                        m2 = m_tiles[c - 1][q // (NQ // 2)]
                        ob = ob_tiles[c - 1]
                        ps = psp.tile([128, 1024], F32, tag="b")
                        mm_pair(ps, w2_sb, m2, q % (NQ // 2))
                        evac(ev2[q % 8], ob[:, bass.ts(q, 1024)], ps[:])
                # transpose each half as soon as its stage-1 evacs land
                if c < nchunks and qq in (NQ // 4 - 1, NQ // 2 - 1):
                    hh = 0 if qq == NQ // 4 - 1 else 1
                    if hh == 0:
                        m_tiles[c] = [m2p.tile([128, HFD], BF16, tag="m2",
                                               name="m2") for _ in range(2)]
                    nc.vector.transpose(m_tiles[c][hh][:].bitcast(I32),
                                        y_tiles[c][hh][:].bitcast(I32))
            if c >= 1:
                nc.sync.dma_start(OUT[:, (c - 1) * FD:c * FD], ob_tiles[c - 1][:])
                m_tiles.pop(c - 1)
                ob_tiles.pop(c - 1)
                y_tiles.pop(c - 1, None)

    if not nc.is_finalized():
        nc.finalize()
    return nc


_NC_CACHE = {}


def _get_nc(ntok):
    if ntok not in _NC_CACHE:
        _NC_CACHE[ntok] = build_nc(ntok)
    return _NC_CACHE[ntok]


def prepare_in_maps(x, w1, w2):
    """Host-side shard + layout transform. Returns (in_maps, ntok)."""
    xf = np.ascontiguousarray(x, dtype=np.float32).reshape(-1, N)
    ntok_total = xf.shape[0]
    assert ntok_total % N_CORES == 0
    ntok = ntok_total // N_CORES

    perm = _perm()
    w1p = _f32_to_bf16_u16(np.ascontiguousarray(w1, np.float32)[:, perm])
    w2p = _f32_to_bf16_u16(np.ascontiguousarray(w2, np.float32)[:, perm])
    w1p = w1p.view(ml_dtypes.bfloat16)
    w2p = w2p.view(ml_dtypes.bfloat16)

    xu = _f32_to_bf16_u16(xf)  # [T, 4096] u16
    in_maps = []
    for i in range(N_CORES):
        xc = xu[i * ntok:(i + 1) * ntok].reshape(ntok // 2, 2, 32, 128)
        # XT[k, tau*64 + 2b + h] = x[2 tau + h, 128 b + k]
        xt = np.ascontiguousarray(xc.transpose(3, 0, 2, 1)).reshape(128, ntok * 32)
        in_maps.append({
            "xt": xt.view(ml_dtypes.bfloat16),
            "w1p": w1p, "w2p": w2p,
        })
    return in_maps, ntok


def postprocess(results, ntok, lead):
    """Gather per-core OT [128, ntok*32] bf16 -> full f32 output."""
    ntok_total = ntok * N_CORES
    out = np.empty((ntok_total, N), np.float32)
    for i in range(N_CORES):
        ot = np.asarray(results[i]["out"]).view(np.uint16)
        # out[2 tau + h, 128 i2 + 32 a2 + b2] = OT[32 a2 + i2, tau*64 + 2 b2 + h]
        ot = ot.reshape(4, 32, ntok // 2, 32, 2)      # [a2, i2, tau, b2, h]
        oc = ot.transpose(2, 4, 1, 0, 3).reshape(ntok, N)
        out[i * ntok:(i + 1) * ntok] = _u16_to_f32(np.ascontiguousarray(oc))
    return out.reshape(*lead, N)


def kernel(x, w1, w2):
    """x [8, 4096, 4096] f32; w1, w2 [128, 128] f32 -> [8, 4096, 4096] f32."""
    lead = x.shape[:-1]
    in_maps, ntok = prepare_in_maps(x, w1, w2)
    nc = _get_nc(ntok)
    res = run_bass_kernel_spmd(nc, in_maps, list(range(N_CORES)))
    return postprocess(res.results, ntok, lead)
